# revision 1
# baseline (speedup 1.0000x reference)
"""Trainium2 Bass kernel for BatchWiseTripletDistanceLoss.

Math: loss = mean_t relu(cos_d(s[a_t], s[p_t]) - cos_d(s[a_t], s[n_t]) + margin)
with cos_d(x, y) = 1 - <x,y>/max(|x||y|, eps).

Cosine distances depend only on (row, row) pairs of the 512x256 sample
matrix, so the kernel computes the 512x512 cosine-SIMILARITY matrix
sim = R S S^T R (R = diag(1/|s_i|)) on-device via TensorE and evaluates
    relu(sim[a,p] - sim[a,n] + margin)          ("1-" cancels in the diff)
on a dense [row, col] grid: each triplet is scattered to grid cell
(a_t, n_t) carrying p_t+1 (gpsimd local_scatter = true per-partition
scatter).  The distinct positives of each row form a small palette
(~13 entries here); palette VALUES are extracted from the sim row by a
second local_scatter, and a short loop over palette slots evaluates
masked relu terms, so no per-triplet gather is ever needed.

Sharding: 8 cores split the grid into (row mod 4) x (column half)
quadrants of [128, 256].  The host only transposes/permutes/buckets/pads
the given arrays (layout + palette metadata, no float math) and sums the
8 partial scalars at the end.
"""
import sys

sys.path.insert(0, "/opt/trn_rl_repo")

from contextlib import ExitStack

import numpy as np
import ml_dtypes

ml_bf16 = ml_dtypes.bfloat16

import concourse.bacc as bacc
import concourse.bass as bass
import concourse.tile as tile
from concourse import mybir
from concourse.bass_utils import run_bass_kernel_spmd

DT = mybir.dt
OP = mybir.AluOpType
ACTF = mybir.ActivationFunctionType

N = 512
D = 256
MARGIN = 0.15
NCORES = 8
LCOL = 256  # columns per core (half)
NROW = 128  # rows per core (stride-4 residue class)
NCHUNK = 4  # main-loop pipeline chunks


def _build_program(s_pad: int):
    """Build + compile the SPMD program (identical for all 8 cores)."""
    nc = bacc.Bacc(
        "TRN2", target_bir_lowering=False, debug=False, num_devices=NCORES
    )
    f32, i32, i16, f16 = DT.float32, DT.int32, DT.int16, DT.float16

    WB = 256 + 512 + 256 + s_pad
    d_packa = nc.dram_tensor("packa", [128, 1280], f32, kind="ExternalInput").ap()
    d_packb = nc.dram_tensor("packb", [NROW, WB], i16, kind="ExternalInput").ap()
    d_out = nc.dram_tensor("out", [NROW, NCHUNK], f32, kind="ExternalOutput").ap()

    with tile.TileContext(nc) as tc, ExitStack() as ctx:
        cpool = ctx.enter_context(tc.tile_pool(name="const", bufs=1))
        wpool = ctx.enter_context(tc.tile_pool(name="work", bufs=2))
        mpool = ctx.enter_context(tc.tile_pool(name="mainloop", bufs=4))
        ppool = ctx.enter_context(tc.tile_pool(name="psum", bufs=2, space="PSUM"))
        pfin = ctx.enter_context(tc.tile_pool(name="psumfin", bufs=1, space="PSUM"))
        pbig = ctx.enter_context(tc.tile_pool(name="psumbig", bufs=1, space="PSUM"))

        # ---- load inputs (two packed DMAs) ------------------------------
        packa = cpool.tile([128, 1280], f32)
        nc.sync.dma_start(packa[:], d_packa)
        packb = cpool.tile([NROW, 256 + 512 + 256 + s_pad], DT.int16)
        nc.scalar.dma_start(packb[:], d_packb)
        st = [packa[:, 0:512], packa[:, 512:1024]]
        sr = [packa[:, 1024:1152], packa[:, 1152:1280]]
        nbuk16 = packb[:, 0:256]
        sidx16 = packb[:, 256:768]
        pbuk16 = packb[:, 768:1024].bitcast(DT.bfloat16)
        ranks1 = packb[:, 1024 : 1024 + s_pad].bitcast(DT.bfloat16)

        ones_col = cpool.tile([128, 1], f32)
        nc.vector.memset(ones_col[:], 1.0)
        ones_row1 = cpool.tile([1, 128], f32)
        nc.vector.memset(ones_row1[:], 1.0)

        # ---- preload ACT sqrt table during the DMA phase ----------------
        dumin = cpool.tile([1, 1], f32)
        nc.vector.memset(dumin[:], 4.0)
        dumout = cpool.tile([1, 1], f32)
        nc.scalar.sqrt(dumout[:], dumin[:])

        # ---- pidxg scatter (independent of samples) ---------------------
        pidxg = wpool.tile([NROW, LCOL], DT.bfloat16, tag="pidxg")
        nc.gpsimd.local_scatter(
            pidxg[:], pbuk16, nbuk16,
            channels=128, num_elems=LCOL, num_idxs=LCOL,
        )

        # ---- squares on DVE (early, one op) -----------------------------
        sqall = wpool.tile([128, 1280], f32, tag="sqall")
        nc.vector.tensor_tensor(sqall[:], packa[:], packa[:], OP.mult)
        sq = [sqall[:, 0:512], sqall[:, 512:1024]]
        sqr = [sqall[:, 1024:1152], sqall[:, 1152:1280]]

        # ---- PE: norm reductions first, then sim, then RB ---------------
        n2p = pbig.tile([1, N], f32, tag="n2row")
        for k in range(2):
            nc.tensor.matmul(n2p[:], ones_col[:], sq[k], start=(k == 0), stop=(k == 1))
        n2rp = ppool.tile([128, 1], f32, tag="n2rp")
        for k in range(2):
            nc.tensor.matmul(n2rp[:], sqr[k], ones_col[:], start=(k == 0), stop=(k == 1))
        simp = pbig.tile([128, N], f32, tag="simp")
        for k in range(2):
            nc.tensor.matmul(simp[:], sr[k], st[k], start=(k == 0), stop=(k == 1))

        nrow = wpool.tile([1, N], f32, tag="nrow")
        nc.scalar.sqrt(nrow[:], n2p[:])
        nrr = wpool.tile([128, 1], f32, tag="nrr")
        nc.scalar.sqrt(nrr[:], n2rp[:])
        # preload relu table right after the sqrts (hidden off critical path)
        durelu = cpool.tile([1, 1], f32)
        nc.scalar.activation(durelu[:], dumout[:], ACTF.Relu)

        rrow = wpool.tile([1, N], f32, tag="rrow")
        rscr = wpool.tile([1, N], f32, tag="rscr")
        nc.vector.reciprocal_approx_accurate(rrow[:], nrow[:], rscr[:])
        rr = cpool.tile([128, 1], f32)
        rscr2 = wpool.tile([128, 1], f32, tag="rscr2")
        nc.vector.reciprocal_approx_accurate(rr[:], nrr[:], rscr2[:])

        rbp = pbig.tile([128, N], f32, tag="rb")
        nc.tensor.matmul(rbp[:], ones_row1[:], rrow[:], start=True, stop=True)
        t0 = wpool.tile([128, N], f32, tag="t0")
        nc.scalar.activation(t0[:], simp[:], ACTF.Copy, scale=rr[:])
        simrow = cpool.tile([128, N], f32)
        nc.vector.tensor_tensor(simrow[:], t0[:], rbp[:], OP.mult)
        sim16 = cpool.tile([128, N], f16)
        nc.scalar.copy(sim16[:], simrow[:])

        # ---- palette values + margin bias -------------------------------
        palv16 = wpool.tile([128, s_pad], f16, tag="palv16")
        nc.gpsimd.local_scatter(
            palv16[:], sim16[:], sidx16,
            channels=128, num_elems=s_pad, num_idxs=N,
        )
        palvf = wpool.tile([128, s_pad], f32, tag="palvf")
        nc.scalar.copy(palvf[:], palv16[:])
        mb = wpool.tile([128, s_pad], f32, tag="mb")
        nc.vector.tensor_scalar(mb[:], palvf[:], -1.0, MARGIN, OP.mult, OP.add)

        # ---- main palette loop (batched, chunked for pipelining) --------
        bounds = [(s_pad * c) // NCHUNK for c in range(NCHUNK + 1)]
        accs = wpool.tile([128, NCHUNK], f32, tag="accs")
        for c in range(NCHUNK):
            lo, hi = bounds[c], bounds[c + 1]
            w = (hi - lo) * LCOL
            msc = mpool.tile([128, w], DT.bfloat16, tag="msc", name=f"msc{c}")
            nc.vector.tensor_tensor(
                msc[:].rearrange("p (s j) -> p s j", s=hi - lo),
                pidxg[:].unsqueeze(1).to_broadcast((NROW, hi - lo, LCOL)),
                ranks1[:, lo:hi].unsqueeze(2).to_broadcast((NROW, hi - lo, LCOL)),
                OP.is_equal,
            )
            t1c = mpool.tile([128, w], f32, tag="t1c", name=f"t1c{c}")
            nc.vector.tensor_tensor(
                t1c[:].rearrange("p (s j) -> p s j", s=hi - lo),
                simrow[:, 0:LCOL].unsqueeze(1).to_broadcast((NROW, hi - lo, LCOL)),
                mb[:, lo:hi].unsqueeze(2).to_broadcast((NROW, hi - lo, LCOL)),
                OP.add,
            )
            mkc = mpool.tile([128, w], f32, tag="mkc", name=f"mkc{c}")
            eng = nc.gpsimd if c < 2 else nc.vector
            eng.tensor_tensor(mkc[:], msc[:], t1c[:], OP.mult)
            rlc = mpool.tile([128, w], f32, tag="rlc", name=f"rlc{c}")
            nc.scalar.activation(
                rlc[:], mkc[:], ACTF.Relu, accum_out=accs[:, c : c + 1]
            )
        nc.sync.dma_start(d_out, accs[:])

    nc.compile()
    return nc


_PROGRAM_CACHE = {}


def _get_program(s_pad):
    if s_pad not in _PROGRAM_CACHE:
        _PROGRAM_CACHE[s_pad] = _build_program(s_pad)
    return _PROGRAM_CACHE[s_pad]


def _shard_inputs(samples, targets, a, p, n, s_pad):
    """Per-core layout: transpose/permute samples, bucket triplets, build
    palette metadata (distinct positives per row)."""
    in_maps = []
    for core in range(NCORES):
        R, H = core >> 1, core & 1
        rows = np.arange(NROW, dtype=np.int64) * 4 + R
        perm = np.concatenate(
            [np.arange(256 * H, 256 * H + 256), np.arange(256 * (1 - H), 256 * (2 - H))]
        )
        sel = ((a & 3) == R) & ((n >> 8) == H)
        asel, psel, nsel = a[sel], p[sel], n[sel]
        q = asel >> 2
        order = np.argsort(q, kind="stable")
        qs = q[order]
        counts = np.bincount(qs, minlength=NROW)
        if counts.max() > LCOL:
            raise ValueError("bucket overflow")
        starts = np.zeros(NROW, dtype=np.int64)
        starts[1:] = np.cumsum(counts)[:-1]
        slot = np.arange(len(qs)) - starts[qs]
        nbuk = np.full((NROW, LCOL), -1, dtype=np.int16)
        nbuk[qs, slot] = (nsel[order] & 255).astype(np.int16)

        # palettes: distinct positives per row; local col of raw id v:
        # (v & 255) + 256 * (v >> 8 != H)
        sidx = np.full((NROW, N), -1, dtype=np.int16)
        palidx1 = np.full((NROW, s_pad), -1.0, dtype=np.float32)  # -1 matches nothing
        rankof = {}
        ar = a[(a & 3) == R]
        pr = p[(a & 3) == R]
        rr_ = ar >> 2
        for qq in range(NROW):
            vals = np.unique(pr[rr_ == qq])
            if len(vals) > s_pad:
                raise ValueError("palette overflow")
            if len(vals) == 0:
                continue
            lcols = (vals & 255) + 256 * ((vals >> 8) != H)
            sidx[qq, lcols] = np.arange(len(vals), dtype=np.int16)
            palidx1[qq, : len(vals)] = vals + 1.0
            for s_, v in enumerate(vals):
                rankof[(qq, v)] = s_ + 1
        # rank+1 of each triplet's positive within its row palette
        pbuk = np.zeros((NROW, LCOL), dtype=np.float32)
        pbuk[qs, slot] = np.array(
            [rankof[(qqv, pv)] for qqv, pv in zip(qs, psel[order])], dtype=np.float32
        )
        pbuk = pbuk.astype(ml_bf16)
        ranks1 = np.broadcast_to(
            np.arange(1, s_pad + 1, dtype=np.float32), (NROW, s_pad)
        ).astype(ml_bf16)
        packa = np.concatenate(
            [
                np.ascontiguousarray(samples[perm].T).reshape(2, 128, N).transpose(1, 0, 2).reshape(128, 1024),
                np.ascontiguousarray(samples[rows].T).reshape(2, 128, NROW).transpose(1, 0, 2).reshape(128, 256),
            ],
            axis=1,
        ).astype(np.float32)
        packb = np.concatenate(
            [
                nbuk.view(np.int16) if nbuk.dtype == np.int16 else nbuk,
                sidx,
                pbuk.view(np.int16),
                ranks1.view(np.int16),
            ],
            axis=1,
        )
        in_maps.append({"packa": packa, "packb": packb})
    return in_maps


def kernel(samples, targets, anchor_idx, pos_idx, neg_idx, _want_trace=False):
    samples = np.asarray(samples, dtype=np.float32)
    targets = np.asarray(targets).astype(np.int32)
    a = np.asarray(anchor_idx).astype(np.int64)
    p = np.asarray(pos_idx).astype(np.int64)
    n = np.asarray(neg_idx).astype(np.int64)
    T = a.shape[0]
    assert samples.shape == (N, D)

    ok = (
        np.all((a >= 0) & (a < N) & (p >= 0) & (p < N) & (n >= 0) & (n < N))
        and len(np.unique(a * N + n)) == T
    )
    if not ok:
        raise NotImplementedError("inputs violate mined-triplet structure")

    ap_pairs = np.unique(a * N + p)
    npal = np.bincount(ap_pairs // N, minlength=N)
    s_max = int(npal.max())
    s_pad = max(2, s_max + (s_max & 1))
    if s_pad > 32:
        raise NotImplementedError("palette too large for this kernel")

    nc = _get_program(s_pad)
    in_maps = _shard_inputs(samples, targets, a, p, n, s_pad)
    res = run_bass_kernel_spmd(nc, in_maps, list(range(NCORES)), trace=_want_trace)
    total = sum(float(res.results[c]["out"].astype(np.float64).sum()) for c in range(NCORES))
    loss = np.float32(total / T)
    if _want_trace:
        return loss, res
    return loss



# revision 6
# speedup vs baseline: 1.7707x; 1.7707x over previous
"""Trainium2 Bass kernel for BatchWiseTripletDistanceLoss.

Math: loss = mean_t relu(cos_d(s[a_t], s[p_t]) - cos_d(s[a_t], s[n_t]) + margin)
with cos_d(x, y) = 1 - <x,y>/max(|x||y|, eps).  The "1-" cancels in the
difference, so with C[q, j] = <s_q, s_j>/(|s_q||s_j|) each triplet term is
relu(C[a,n] - C[a,p] + margin).

Device algorithm (per core; grid = 128 anchor rows x 256 negative columns):
  - sim = (R S) (S^T R') via TensorE on an f16 copy of samples whose column
    order is a per-core permutation `perm` placing the core's own 128 rows
    at positions 192:320 (so the matmul's stationary operand and the row
    norms are slices of the same tensor) and the core's negative half at
    positions 0:256.
  - Triplets of each row are bucketed host-side, sorted by positive id, so
    equal-positive triplets form contiguous runs of slots.  One gpsimd
    local_scatter drops C[a,p] at each run start (`vgrid`), a second drops
    C[a,n] at each triplet's slot (`buk`).  A DVE prefix scan
        state = keep * state - vgrid     (keep = 0 at run starts, 1 inside)
    forward-fills -C[a,p] across each run; a sentinel value +BIG scattered
    just past the last slot poisons the empty tail (relu -> 0).
  - loss terms = Relu(buk + scan + margin) summed per partition by one
    Scalar activation with accumulate; host sums 8x128 partials / T.

Host does layout/indexing only (permutations, bucketing, run starts);
all floating-point math runs on device.

Sharding: 8 cores = (anchor row mod 4) x (negative column half).
"""
import sys

sys.path.insert(0, "/opt/trn_rl_repo")

from contextlib import ExitStack

import numpy as np

import concourse.bacc as bacc
import concourse.bass as bass
import concourse.tile as tile
from concourse import mybir
from concourse.bass_utils import run_bass_kernel_spmd

DT = mybir.dt
OP = mybir.AluOpType
ACTF = mybir.ActivationFunctionType

N = 512
D = 256
MARGIN = 0.15
NCORES = 8
NROW = 128  # anchor rows per core
LCOL = 256  # negative columns per core
BIG = 60000.0  # f16-safe sentinel
WPB = 256 + 514 + 256  # nidx | sidx2 | keep


def _build_program():
    nc = bacc.Bacc(
        "TRN2", target_bir_lowering=False, debug=False, num_devices=NCORES
    )
    f32, i16, f16 = DT.float32, DT.int16, DT.float16

    d_packa = nc.dram_tensor("packa", [128, 1024], f16, kind="ExternalInput").ap()
    d_packb = nc.dram_tensor("packb", [NROW, WPB], i16, kind="ExternalInput").ap()
    d_out = nc.dram_tensor("out", [NROW, 1], f32, kind="ExternalOutput").ap()

    with tile.TileContext(nc) as tc, ExitStack() as ctx:
        cpool = ctx.enter_context(tc.tile_pool(name="const", bufs=1))
        wpool = ctx.enter_context(tc.tile_pool(name="work", bufs=2))
        ppool = ctx.enter_context(tc.tile_pool(name="psum", bufs=2, space="PSUM"))
        pbig = ctx.enter_context(tc.tile_pool(name="psumbig", bufs=1, space="PSUM"))

        # ---- inputs (two packed DMAs on separate queues) ----------------
        st = cpool.tile([128, 1024], f16)
        nc.sync.dma_start(st[:], d_packa)
        pb = cpool.tile([NROW, WPB], i16)
        nc.scalar.dma_start(pb[:], d_packb)
        nidx = pb[:, 0:256]
        sidx2 = pb[:, 256:770]
        keepg = pb[:, 770:1026].bitcast(f16)

        ones_col = cpool.tile([128, 1], f16)
        nc.vector.memset(ones_col[:], 1.0)
        ones_row = cpool.tile([1, 128], f16)
        nc.vector.memset(ones_row[:], 1.0)
        one_mov = cpool.tile([1, 1], f16)
        nc.vector.memset(one_mov[:], 1.0)

        # preload ACT tables (Rsqrt, Relu) during the DMA dead time
        dumin = cpool.tile([1, 1], f32)
        nc.vector.memset(dumin[:], 4.0)
        dum1 = cpool.tile([1, 1], f32)
        nc.scalar.activation(dum1[:], dumin[:], ACTF.Abs_reciprocal_sqrt)
        mbias = cpool.tile([128, 1], f32)
        nc.vector.memset(mbias[:], MARGIN)
        dum2 = cpool.tile([1, 1], f32)
        nc.scalar.activation(dum2[:], dum1[:], ACTF.Relu, bias=mbias[0:1, :])

        # ---- squares -> column norms ------------------------------------
        sq = wpool.tile([128, 1024], f16, tag="sq")
        nc.vector.tensor_tensor(sq[:], st[:], st[:], OP.mult)
        n2p = ppool.tile([1, N], f32, tag="n2p")
        for k in range(2):
            nc.tensor.matmul(
                n2p[:], ones_col[:], sq[:, 512 * k : 512 * k + 512],
                start=(k == 0), stop=(k == 1),
            )
        rrow16 = wpool.tile([1, N], f16, tag="rrow16")
        nc.scalar.activation(rrow16[:], n2p[:], ACTF.Abs_reciprocal_sqrt)

        # ---- sim matrix (own rows x all columns) ------------------------
        simp = pbig.tile([128, N], f32, tag="simp")
        for k in range(2):
            nc.tensor.matmul(
                simp[:], st[:, 512 * k + 192 : 512 * k + 320],
                st[:, 512 * k : 512 * k + 512],
                start=(k == 0), stop=(k == 1),
            )

        # rbp[q, j] = rrow[j]; rrp[q, 0] = rrow[192 + q] (own-row rsqrt)
        rbp = pbig.tile([128, N], f32, tag="rbp")
        nc.tensor.matmul(rbp[:], ones_row[:], rrow16[:], start=True, stop=True)
        rrp = ppool.tile([128, 1], f32, tag="rrp")
        nc.tensor.matmul(rrp[:], rrow16[0:1, 192:320], one_mov[:], start=True, stop=True)
        rr_sb = cpool.tile([128, 1], f32)
        nc.vector.tensor_scalar(rr_sb[:], rrp[:], 1.0, 0.0, OP.mult, OP.add)

        # ---- C16 = rr * simp * rrow  (cosine sim, f16, + sentinel cols) --
        C16 = cpool.tile([128, 514], f16)
        nc.vector.memset(C16[:, 512:514], BIG)
        t0 = wpool.tile([128, N], f32, tag="t0")
        for h in range(2):
            cs = slice(256 * h, 256 * h + 256)
            nc.scalar.activation(t0[:, cs], simp[:, cs], ACTF.Copy, scale=rr_sb[:])
            nc.vector.tensor_tensor(C16[:, cs], t0[:, cs], rbp[:, cs], OP.mult)

        # ---- bucket scatters (gpsimd) -----------------------------------
        buk = wpool.tile([NROW, LCOL], f16, tag="buk")
        nc.gpsimd.local_scatter(
            buk[:], C16[:, 0:256], nidx, channels=128, num_elems=LCOL, num_idxs=256
        )
        vgrid = wpool.tile([NROW, LCOL], f16, tag="vgrid")
        nc.gpsimd.local_scatter(
            vgrid[:], C16[:], sidx2, channels=128, num_elems=LCOL, num_idxs=514
        )

        # ---- forward-fill -C[a,p] across runs, add, relu, accumulate ----
        biasg = wpool.tile([NROW, LCOL], f32, tag="biasg")
        nc.vector.tensor_tensor_scan(
            biasg[:], keepg, vgrid[:], 0.0, OP.mult, OP.subtract
        )
        y = wpool.tile([NROW, LCOL], f32, tag="y")
        nc.vector.tensor_tensor(y[:], buk[:], biasg[:], OP.add)
        acc = wpool.tile([NROW, 1], f32, tag="acc")
        rl = wpool.tile([NROW, LCOL], f32, tag="rl")
        nc.scalar.activation(
            rl[:], y[:], ACTF.Relu, bias=mbias[:], accum_out=acc[:]
        )
        nc.sync.dma_start(d_out, acc[:])

    nc.compile()
    return nc


_PROGRAM = None


def _get_program():
    global _PROGRAM
    if _PROGRAM is None:
        _PROGRAM = _build_program()
    return _PROGRAM


def _shard_inputs(samples, a, p, n):
    """Per-core layout: permute samples, bucket triplets (sorted by positive
    id so equal-positive slots are contiguous runs), build scatter indices."""
    in_maps = []
    allr = np.arange(N, dtype=np.int64)
    for core in range(NCORES):
        R, H = core >> 1, core & 1
        inH = (allr >> 8) == H
        ownm = (allr & 3) == R
        own_H = allr[ownm & inH]          # 64
        own_O = allr[ownm & ~inH]         # 64
        non_own_H = allr[~ownm & inH]     # 192
        non_own_O = allr[~ownm & ~inH]    # 192
        perm = np.concatenate([non_own_H, own_H, own_O, non_own_O])
        colpos = np.empty(N, dtype=np.int64)
        colpos[perm] = np.arange(N)
        rows_core = np.concatenate([own_H, own_O])  # partition q -> global row
        qof = np.full(N, -1, dtype=np.int64)
        qof[rows_core] = np.arange(NROW)

        sel = ((a & 3) == R) & ((n >> 8) == H)
        asel, psel, nsel = a[sel], p[sel], n[sel]
        q = qof[asel]
        order = np.lexsort((psel, q))
        qs, ps, ns = q[order], psel[order], nsel[order]
        counts = np.bincount(qs, minlength=NROW)
        starts = np.zeros(NROW, dtype=np.int64)
        starts[1:] = np.cumsum(counts)[:-1]
        slot = np.arange(len(qs)) - starts[qs]  # slot within row (sorted by p)

        nidx = np.full((NROW, 256), -1, dtype=np.int16)
        nidx[qs, colpos[ns]] = slot.astype(np.int16)

        # run starts: first slot of each (q, p) group
        if len(qs):
            newrun = np.ones(len(qs), dtype=bool)
            newrun[1:] = (qs[1:] != qs[:-1]) | (ps[1:] != ps[:-1])
        else:
            newrun = np.zeros(0, dtype=bool)
        sidx2 = np.full((NROW, 514), -1, dtype=np.int16)
        sidx2[qs[newrun], colpos[ps[newrun]]] = slot[newrun].astype(np.int16)
        # sentinel just past the last slot (poisons the empty tail)
        has_room = counts < 256
        sidx2[has_room, 512] = counts[has_room].astype(np.int16)

        keep = np.ones((NROW, 256), dtype=np.float16)
        keep[qs[newrun], slot[newrun]] = 0.0
        keep[has_room, np.minimum(counts, 255)[has_room]] = 0.0

        A16 = samples[perm].astype(np.float16)  # [512, 256]
        AT = np.ascontiguousarray(A16.T)  # [256, 512] = (d, col)
        packa = np.concatenate([AT[0:128], AT[128:256]], axis=1)  # [128, 1024]
        packb = np.concatenate(
            [nidx, sidx2, keep.view(np.int16)], axis=1
        )
        in_maps.append({"packa": packa, "packb": packb})
    return in_maps


def kernel(samples, targets, anchor_idx, pos_idx, neg_idx, _want_trace=False):
    samples = np.asarray(samples, dtype=np.float32)
    a = np.asarray(anchor_idx).astype(np.int64)
    p = np.asarray(pos_idx).astype(np.int64)
    n = np.asarray(neg_idx).astype(np.int64)
    T = a.shape[0]
    assert samples.shape == (N, D)

    ok = (
        np.all((a >= 0) & (a < N) & (p >= 0) & (p < N) & (n >= 0) & (n < N))
        and len(np.unique(a * N + n)) == T
    )
    if not ok:
        raise NotImplementedError("inputs violate mined-triplet structure")

    nc = _get_program()
    in_maps = _shard_inputs(samples, a, p, n)
    res = run_bass_kernel_spmd(nc, in_maps, list(range(NCORES)), trace=_want_trace)
    total = sum(
        float(res.results[c]["out"].astype(np.float64).sum()) for c in range(NCORES)
    )
    loss = np.float32(total / T)
    if _want_trace:
        return loss, res
    return loss
